# revision 3
# baseline (speedup 1.0000x reference)
"""Trainium2 Bass kernel for BaselineProtonet (retrieval_knn).

logits[q, c] = -||query_q - proto_c||_2
  proto_c = mean of 64 support embeddings of class c
  embeddings_stacked: [64 classes * (64 support + 64 query), 1024] f32

Sharding (8 cores): 2D grid, 4 query-groups x 2 class-groups. Core
(a, b) owns query rows 1024a..1024(a+1) and classes 32b..32b+32.
Per-core wire traffic: 1 MB queries (fp8) + 2 MB support (fp8) + 1 KB
one-hot = 3 MB (vs 5.25 MB for the 1D query-shard baseline). The logits
tile computed per core is [32 classes, 1024 queries]; the host stitches
the 4x2 grid (layout/encoding only, no host arithmetic).

Per core:
  protos  : support is the STATIONARY matmul operand (fp8 DoubleRow,
            chunk-pair jp contracts 256 support rows = 4 classes), the
            tiny one-hot streams -> P^T lands directly as [d, class] in
            PSUM (pt), so there is no evacuate->transpose chain on the
            post-DMA critical path.
  W       : ACT scale pt * (-1/32) -> fp8 [128, 8, 32]  (= -2 * P^T / 64)
  ||p||^2 : psq = Square(W) (fp8), contracted with ones on PE -> [32,1]
            f32, scaled 0.25 -> sqrt bias (per-partition = per-class)
  ||q||^2 : per-d-chunk squares split across DVE/ACT/Pool engines as the
            query stream lands; summed over d by all-ones DoubleRow
            matmuls straight into the Gram PSUM group
  Gram    : 8 fp8 DoubleRow matmuls lhsT=W d-pair, rhs=Q^T d-pair
  logits  : -sqrt(dist^2) via ACT sqrt(+bias) and DVE negate, two
            pipelined query halves; output [32, 1024] f32 class-major.
DMAs are spread across all four DMA-capable queues (scalar/vector/
gpsimd/sync) so descriptor generation (~0.6us per dma_start per queue)
overlaps; support pieces are issued ahead of query pieces on each
queue. PE is pre-warmed with dummy matmuls (HAM clock gate) and the
sqrt/square ACT tables are preloaded by dummy activations.
"""

import numpy as np

C = 64          # classes
S = 64          # support per class (== queries per class)
D = 1024        # embedding dim
NCORES = 8
AQ = 4          # query groups
BC = 2          # class groups
CL = C // BC                # 32 classes per core
QL = (C * S) // AQ          # 1024 query rows per core
DCH = D // 128              # 8 d-chunks
SCH = (CL * S) // 128       # 16 support row chunks per core
NJP = SCH // 2              # 8 DoubleRow chunk-pairs

# which engine squares query d-chunk k: 0=DVE, 1=ACT, 2=Pool
SQ_ENG = [0, 0, 0, 0, 1, 2, 0, 2]

_CACHE = {}


def _emit(nc, tc, sup, qt, oh_in, out):
    """Emit the per-core tile program.

    sup:   [128, SCH*D] fp8 DRAM  (support of this core's 32 classes,
                                   swizzled: row p of chunk j holds
                                   support row j*128+p)
    qt:    [128, DCH*QL] fp8 DRAM (queries, swizzled feature-major)
    oh_in: [128, 2*4] fp8 DRAM    (DoubleRow one-hot, shared by pairs)
    out:   [CL, QL] f32 DRAM      (negated distances, class-major)
    """
    from concourse import mybir

    f32 = mybir.dt.float32
    bf16 = mybir.dt.bfloat16
    fp8 = mybir.dt.float8e4
    AF = mybir.ActivationFunctionType

    with (
        tc.tile_pool(name="sb", bufs=1) as sb,
        tc.tile_pool(name="ps", bufs=1, space="PSUM") as ps,
    ):
        # warm the PE clock first-thing (HAM gate needs ~3us of busy
        # before the real matmuls; deps are a single DVE memset)
        wm_in = sb.tile([128, 512], bf16)
        nc.vector.memset(wm_in[:], 0.0)
        wm_ps = ps.tile([128, 512], f32)
        for _ in range(7):
            nc.tensor.matmul(
                wm_ps[:], wm_in[:, 0:128], wm_in[:], start=True, stop=True
            )

        # ---------------- input DMAs: support pieces first on each
        # queue, then a query piece; 4 queues in parallel -------------
        s8 = sb.tile([128, SCH, D], fp8)
        q16 = sb.tile([128, DCH, QL], fp8)
        oh4 = sb.tile([128, 2, 4], fp8)

        def sup_piece(eng, j):
            eng.dma_start(
                s8[:, 2 * j : 2 * (j + 1)],
                sup[:, 2 * j * D : 2 * (j + 1) * D].rearrange(
                    "p (c d) -> p c d", c=2
                ),
            )

        def q_piece(eng, h):
            eng.dma_start(
                q16[:, 2 * h : 2 * (h + 1)],
                qt[:, 2 * h * QL : 2 * (h + 1) * QL].rearrange(
                    "p (k q) -> p k q", k=2
                ),
            )

        sup_piece(nc.scalar, 0)
        sup_piece(nc.scalar, 1)
        sup_piece(nc.scalar, 2)
        q_piece(nc.scalar, 0)
        sup_piece(nc.gpsimd, 3)
        sup_piece(nc.gpsimd, 4)
        q_piece(nc.gpsimd, 1)
        nc.sync.dma_start(oh4[:], oh_in[:, :].rearrange("p (o c) -> p o c", o=2))
        sup_piece(nc.sync, 5)
        sup_piece(nc.sync, 6)
        sup_piece(nc.sync, 7)
        q_piece(nc.sync, 2)
        q_piece(nc.sync, 3)

        # ---------------- constants (DVE: it cannot issue DMAs) ---------
        ones8 = sb.tile([128, 2, CL], fp8)
        nc.vector.memset(ones8[:], 1.0)
        ones1 = sb.tile([128, 2, 1], fp8)
        nc.vector.memset(ones1[:], 1.0)

        # preload the sqrt+square ACT tables off the critical path
        warm_sq = sb.tile([1, 2], f32)
        nc.vector.memset(warm_sq[:], 1.0)
        nc.scalar.activation(warm_sq[:, 0:1], warm_sq[:, 0:1], AF.Sqrt)
        nc.scalar.activation(warm_sq[:, 1:2], warm_sq[:, 1:2], AF.Square)

        # ---------------- prototypes, direct [d, class] layout ----------
        # chunk-pair jp holds 256 support rows = classes 4jp..4jp+4;
        # support chunk is the stationary operand, one-hot streams.
        s8v = s8[:].rearrange("p (jp o) d -> p jp o d", o=2)
        pt_ps = ps.tile([128, DCH, 4 * NJP], f32)
        for jp in range(NJP):
            for k in range(DCH):
                nc.tensor.matmul(
                    pt_ps[:, k, 4 * jp : 4 * (jp + 1)],
                    s8v[:, jp, :, 128 * k : 128 * (k + 1)],
                    oh4[:],
                    start=True,
                    stop=True,
                    perf_mode=mybir.MatmulPerfMode.DoubleRow,
                )

        # ---------------- ||q||^2 squares (per chunk, 3 engines) --------
        qsq = sb.tile([128, DCH, QL], fp8)
        for k in range(DCH):
            e = SQ_ENG[k]
            if e == 0:
                nc.vector.tensor_mul(qsq[:, k], q16[:, k], q16[:, k])
            elif e == 1:
                nc.scalar.square(qsq[:, k], q16[:, k])
            else:
                nc.gpsimd.tensor_mul(qsq[:, k], q16[:, k], q16[:, k])

        # ||q||^2 matmuls open the s_ps PSUM groups and track the query
        # stream; the Gram matmuls close them after W is ready.
        # s_ps[c, q] = sum_dp ( ones^T qsq_dp + W_dp^T q_dp )
        #            = ||q||^2 - 2 q.p
        s_ps = ps.tile([CL, QL], f32)
        qsqv = qsq[:].rearrange("p (dp o) q -> p dp o q", o=2)
        q16v = q16[:].rearrange("p (dp o) q -> p dp o q", o=2)
        for h in range(2):
            for dp in range(DCH // 2):
                nc.tensor.matmul(
                    s_ps[:, 512 * h : 512 * (h + 1)],
                    ones8[:],
                    qsqv[:, dp, :, 512 * h : 512 * (h + 1)],
                    start=(dp == 0),
                    stop=False,
                    perf_mode=mybir.MatmulPerfMode.DoubleRow,
                )

        # ---------------- W = -2 * P^T / 64 (fp8, ACT scale) ------------
        W = sb.tile([128, DCH, CL], fp8)
        nc.scalar.mul(W[:], pt_ps[:], -1.0 / 32.0)

        # ||p||^2 via psq = W^2 on ACT, ones-contraction on PE, x0.25
        psq = sb.tile([128, DCH, CL], fp8)
        nc.scalar.square(psq[:], W[:])
        psqv = psq[:].rearrange("p (dp o) c -> p dp o c", o=2)
        pn_ps = ps.tile([CL, 1], f32)
        for dp in range(DCH // 2):
            nc.tensor.matmul(
                pn_ps[:],
                psqv[:, dp],
                ones1[:],
                start=(dp == 0),
                stop=(dp == DCH // 2 - 1),
                perf_mode=mybir.MatmulPerfMode.DoubleRow,
            )
        pn_col = sb.tile([CL, 1], f32)
        nc.scalar.mul(pn_col[:], pn_ps[:], 0.25)

        # ------- Gram matmuls (close the s_ps groups) --------------------
        Wv = W[:].rearrange("p (dp o) c -> p dp o c", o=2)
        for h in range(2):
            for dp in range(DCH // 2):
                nc.tensor.matmul(
                    s_ps[:, 512 * h : 512 * (h + 1)],
                    Wv[:, dp],
                    q16v[:, dp, :, 512 * h : 512 * (h + 1)],
                    start=False,
                    stop=(dp == DCH // 2 - 1),
                    perf_mode=mybir.MatmulPerfMode.DoubleRow,
                )

        # ------- sqrt(+||p||^2), negate, store (2 q-halves pipelined) ----
        lt = sb.tile([CL, QL], f32)
        for h in range(2):
            s = slice(512 * h, 512 * (h + 1))
            nc.scalar.activation(lt[:, s], s_ps[:, s], AF.Sqrt, bias=pn_col[:, 0:1])
            nc.vector.tensor_scalar_mul(lt[:, s], lt[:, s], -1.0)
            nc.sync.dma_start(out[:, s], lt[:, s])


def _build():
    if "nc" in _CACHE:
        return _CACHE["nc"]
    from concourse import bacc, mybir, tile

    f32 = mybir.dt.float32
    fp8 = mybir.dt.float8e4
    nc = bacc.Bacc(
        "TRN2",
        target_bir_lowering=False,
        debug=False,
        enable_asserts=False,
        num_devices=NCORES,
    )
    sup = nc.dram_tensor("sup", [128, SCH * D], fp8, kind="ExternalInput").ap()
    qt = nc.dram_tensor("qt", [128, DCH * QL], fp8, kind="ExternalInput").ap()
    oh_in = nc.dram_tensor("oh", [128, 2 * 4], fp8, kind="ExternalInput").ap()
    out = nc.dram_tensor("out", [CL, QL], f32, kind="ExternalOutput").ap()
    with tile.TileContext(nc) as tc:
        _emit(nc, tc, sup, qt, oh_in, out)
    nc.compile()
    _CACHE["nc"] = nc
    return nc


def _onehot():
    import ml_dtypes

    # DoubleRow one-hot: oh4[p, o, c] = 1 iff local class c owns support
    # row o*128+p of a chunk pair, i.e. c == 2*o + p//64
    p = np.arange(128)[:, None, None]
    o = np.arange(2)[None, :, None]
    c = np.arange(4)[None, None, :]
    oh = (c == 2 * o + p // 64).astype(ml_dtypes.float8_e4m3)
    return np.ascontiguousarray(oh.reshape(128, 8))


def _shard(embeddings):
    import ml_dtypes

    emb = np.asarray(embeddings, dtype=np.float32).reshape(C, 2 * S, D)
    oh = _onehot()
    # support per class-group b: [CL*S, D] -> swizzled [128, SCH, D]
    sups = []
    for b in range(BC):
        sb = emb[CL * b : CL * (b + 1), :S, :].reshape(SCH, 128, D)
        sb = sb.transpose(1, 0, 2)
        sups.append(
            np.ascontiguousarray(
                sb.astype(ml_dtypes.float8_e4m3).reshape(128, SCH * D)
            )
        )
    # queries per query-group a: Q^T swizzled [128, DCH, QL]
    query_set = emb[:, S:, :].reshape(C * S, D)
    qts = []
    for a in range(AQ):
        q = query_set[QL * a : QL * (a + 1)]
        qt_a = q.T.reshape(DCH, 128, QL).transpose(1, 0, 2)
        qts.append(
            np.ascontiguousarray(
                qt_a.astype(ml_dtypes.float8_e4m3).reshape(128, DCH * QL)
            )
        )
    in_maps = []
    for i in range(NCORES):
        a, b = divmod(i, BC)
        in_maps.append({"sup": sups[b], "qt": qts[a], "oh": oh})
    return in_maps


def _gather(outs):
    """Stitch per-core [CL, QL] blocks into full [C*S, C] logits."""
    logits = np.empty((C * S, C), dtype=np.float32)
    for i in range(NCORES):
        a, b = divmod(i, BC)
        logits[QL * a : QL * (a + 1), CL * b : CL * (b + 1)] = (
            np.asarray(outs[i], dtype=np.float32).T
        )
    return logits


def kernel(embeddings_stacked, n_classes, n_support, **_unused):
    assert int(n_classes) == C and int(n_support) == S
    emb = np.asarray(embeddings_stacked)
    assert emb.shape == (C * 2 * S, D), emb.shape

    from concourse import bass_utils

    nc = _build()
    in_maps = _shard(emb)
    try:
        res = bass_utils.run_bass_kernel_spmd(
            nc, in_maps, core_ids=list(range(NCORES))
        )
    except Exception:
        # transient device/runtime hiccups have been observed; retry once
        res = bass_utils.run_bass_kernel_spmd(
            nc, in_maps, core_ids=list(range(NCORES))
        )
    return _gather([res.results[i]["out"] for i in range(NCORES)])


if __name__ == "__main__":
    rng = np.random.default_rng(0)
    emb = rng.standard_normal((C * 2 * S, D), dtype=np.float32)
    got = kernel(emb, C, S)
    print("kernel output", got.shape, got.dtype)


# revision 4
# speedup vs baseline: 1.1163x; 1.1163x over previous
"""Trainium2 Bass kernel for BaselineProtonet (retrieval_knn).

logits[q, c] = -||query_q - proto_c||_2
  proto_c = mean of 64 support embeddings of class c
  embeddings_stacked: [64 classes * (64 support + 64 query), 1024] f32

Sharding (8 cores): 2D grid, 4 query-groups x 2 class-groups. Core
(a, b) owns query rows 1024a..1024(a+1) and classes 32b..32b+32.
Per-core wire traffic: 2 MB queries (bf16) + 2 MB support (fp8) + 8 KB
one-hot = 4 MB (vs 5.25 MB for the 1D query-shard baseline), and the
prototype matmul work halves. The per-core logits tile is [32 classes,
1024 queries]; the host stitches the 4x2 grid (layout/encoding only).

Per core:
  protos  : 16 fp8 DoubleRow one-hot matmuls (one-hot stationary per
            chunk-pair, support streams 512 cols) -> p_ps [32, 1024] f32
  W       : ACT evac p_ps/64 -> bf16, 8 PE transposes, ACT scale -2
            -> W bf16 [128 d, 8, 32 c]
  ||p||^2 : ACT square-accumulate on the evacuated protos -> [32,1] f32,
            summed on DVE -> sqrt bias (per-partition = per-class)
  ||q||^2 : bf16 DVE squares per d-chunk tracking the query stream;
            summed over d by all-ones matmuls into the Gram PSUM group
  Gram    : 16 bf16 matmuls lhsT=W chunk (load hidden by 512-col
            stream), rhs=Q^T chunk
  logits  : -sqrt(dist^2) via ACT sqrt(+bias) and DVE negate, two
            pipelined query halves; output [32, 1024] f32 class-major.
DMAs are spread across the three DMA-capable queues (scalar/sync HWDGE,
gpsimd SWDGE); the one-hot rides at the head of the support tensor. PE
is pre-warmed with dummy matmuls (HAM clock gate) and the sqrt/square
ACT tables are preloaded by dummy activations. PE program order places
query-gated matmuls last so the in-order engine queue never blocks the
prototype/Gram chain on late query chunks.
"""

import numpy as np

C = 64          # classes
S = 64          # support per class (== queries per class)
D = 1024        # embedding dim
NCORES = 8
AQ = 4          # query groups
BC = 2          # class groups
CL = C // BC                # 32 classes per core
QL = (C * S) // AQ          # 1024 query rows per core
DCH = D // 128              # 8 d-chunks
SCH = (CL * S) // 128       # 16 support row chunks per core
NJP = SCH // 2              # 8 DoubleRow chunk-pairs
OHW = NJP * 2 * CL          # one-hot words (512 fp8 = 4 cols of f32)

_CACHE = {}


def _emit(nc, tc, sup, qt, out):
    """Emit the per-core tile program.

    sup: [128, OHW + SCH*D] fp8 DRAM (one-hot header + support of this
         core's 32 classes, swizzled: row p of chunk j = support row
         j*128+p)
    qt:  [128, DCH*QL] bf16 DRAM    (queries, swizzled feature-major)
    out: [CL, QL] f32 DRAM          (negated distances, class-major)
    """
    from concourse import masks, mybir

    f32 = mybir.dt.float32
    bf16 = mybir.dt.bfloat16
    fp8 = mybir.dt.float8e4
    AF = mybir.ActivationFunctionType

    with (
        tc.tile_pool(name="sb", bufs=1) as sb,
        tc.tile_pool(name="ps", bufs=1, space="PSUM") as ps,
    ):
        # warm the PE clock first-thing (HAM gate needs ~3us of busy
        # before the real matmuls; deps are a single DVE memset)
        wm_in = sb.tile([128, 512], bf16)
        nc.vector.memset(wm_in[:], 0.0)
        wm_ps = ps.tile([128, 512], f32)
        for _ in range(7):
            nc.tensor.matmul(
                wm_ps[:], wm_in[:, 0:128], wm_in[:], start=True, stop=True
            )

        # ---------------- input DMAs --------------------------------
        # sc8 = one-hot header + support; pieces of 4 chunks (512 KB)
        sc8 = sb.tile([128, OHW + SCH * D], fp8)
        q16 = sb.tile([128, DCH, QL], bf16)

        def sup_piece(eng, lo, hi):
            eng.dma_start(sc8[:, lo:hi], sup[:, lo:hi])

        def q_piece(eng, h):
            eng.dma_start(
                q16[:, 4 * h : 4 * (h + 1)],
                qt[:, 4 * h * QL : 4 * (h + 1) * QL].rearrange(
                    "p (k q) -> p k q", k=4
                ),
            )

        E = OHW
        sup_piece(nc.scalar, 0, E + 4 * D)          # one-hot + chunks 0-3
        sup_piece(nc.scalar, E + 4 * D, E + 8 * D)  # chunks 4-7
        q_piece(nc.gpsimd, 0)                       # query chunks 0-3
        sup_piece(nc.sync, E + 8 * D, E + 12 * D)   # chunks 8-11
        sup_piece(nc.sync, E + 12 * D, E + 16 * D)  # chunks 12-15
        q_piece(nc.sync, 1)                         # query chunks 4-7

        oh = sc8[:, 0:OHW].rearrange("p (jp o c) -> p jp o c", jp=NJP, o=2)
        s8v = sc8[:, OHW:].rearrange("p (jp o d) -> p jp o d", jp=NJP, o=2)

        # ---------------- constants (DVE: it cannot issue DMAs) ------
        ident = sb.tile([128, 128], bf16)
        masks.make_identity(nc, ident[:])
        ones16 = sb.tile([128, CL], bf16)
        nc.vector.memset(ones16[:], 1.0)

        # preload the sqrt+square ACT tables off the critical path
        warm_sq = sb.tile([1, 2], f32)
        nc.vector.memset(warm_sq[:], 1.0)
        nc.scalar.activation(warm_sq[:, 0:1], warm_sq[:, 0:1], AF.Sqrt)
        nc.scalar.activation(warm_sq[:, 1:2], warm_sq[:, 1:2], AF.Square)

        # ---------------- prototypes [class, d] ----------------------
        # chunk-pair jp = 256 support rows = classes 4jp..4jp+4; one-hot
        # is stationary, support streams; fp8 DoubleRow.
        p_ps = ps.tile([CL, D], f32)
        for jp in range(NJP):
            for h in range(2):
                nc.tensor.matmul(
                    p_ps[:, 512 * h : 512 * (h + 1)],
                    oh[:, jp],
                    s8v[:, jp, :, 512 * h : 512 * (h + 1)],
                    start=(jp == 0),
                    stop=(jp == NJP - 1),
                    perf_mode=mybir.MatmulPerfMode.DoubleRow,
                )

        # ---------------- ||q||^2 squares (DVE, bf16 2x) -------------
        qsq = sb.tile([128, DCH, QL], bf16)
        for k in range(DCH):
            nc.vector.tensor_mul(qsq[:, k], q16[:, k], q16[:, k])

        # early ||q||^2 matmuls open the two s_ps PSUM bank groups and
        # track the first query piece while the W chain completes
        s_ps = ps.tile([CL, QL], f32)
        for h in range(2):
            for k in range(4):
                nc.tensor.matmul(
                    s_ps[:, 512 * h : 512 * (h + 1)],
                    ones16[:],
                    qsq[:, k, 512 * h : 512 * (h + 1)],
                    start=(k == 0),
                    stop=False,
                )

        # ---------------- W chain (ACT + PE transposes) --------------
        # evacuate p/64 in two halves (separate tiles so the transposes
        # can start on half A while half B evacuates)
        psbA = sb.tile([CL, 512], bf16)
        psbB = sb.tile([CL, 512], bf16)
        nc.scalar.mul(psbA[:], p_ps[:, 0:512], 1.0 / S)
        nc.scalar.mul(psbB[:], p_ps[:, 512:1024], 1.0 / S)

        ptp = ps.tile([128, DCH, CL], bf16)
        for k in range(DCH):
            half = psbA if k < 4 else psbB
            nc.tensor.transpose(
                ptp[:, k],
                half[:, 128 * (k % 4) : 128 * (k % 4 + 1)],
                ident[0:CL, 0:CL],
            )
        W = sb.tile([128, DCH, CL], bf16)
        nc.scalar.mul(W[:], ptp[:], -2.0)

        # ||p||^2 via ACT square-accumulate on the evacuated protos
        pn_dump = sb.tile([CL, D], bf16)
        pnA = sb.tile([CL, 1], f32)
        pnB = sb.tile([CL, 1], f32)
        pn_col = sb.tile([CL, 1], f32)
        nc.scalar.activation(pn_dump[:, 0:512], psbA[:], AF.Square, accum_out=pnA[:])
        nc.scalar.activation(pn_dump[:, 512:1024], psbB[:], AF.Square, accum_out=pnB[:])
        nc.vector.tensor_add(pn_col[:], pnA[:], pnB[:])

        # ---------------- Gram + late ||q||^2 ------------------------
        # Gram k tracks query chunk k (W load hidden by 512-col stream);
        # the k>=4 ||q||^2 matmuls close the groups after the last
        # squares land.
        for h in range(2):
            for k in range(DCH):
                nc.tensor.matmul(
                    s_ps[:, 512 * h : 512 * (h + 1)],
                    W[:, k],
                    q16[:, k, 512 * h : 512 * (h + 1)],
                    start=False,
                    stop=False,
                )
        for h in range(2):
            for k in range(4, DCH):
                nc.tensor.matmul(
                    s_ps[:, 512 * h : 512 * (h + 1)],
                    ones16[:],
                    qsq[:, k, 512 * h : 512 * (h + 1)],
                    start=False,
                    stop=(k == DCH - 1),
                )

        # ------- sqrt(+||p||^2), negate, store (2 halves pipelined) --
        lt = sb.tile([CL, QL], f32)
        for h in range(2):
            s = slice(512 * h, 512 * (h + 1))
            nc.scalar.activation(lt[:, s], s_ps[:, s], AF.Sqrt, bias=pn_col[:, 0:1])
            nc.vector.tensor_scalar_mul(lt[:, s], lt[:, s], -1.0)
            nc.scalar.dma_start(out[:, s], lt[:, s])


def _build():
    if "nc" in _CACHE:
        return _CACHE["nc"]
    from concourse import bacc, mybir, tile

    f32 = mybir.dt.float32
    bf16 = mybir.dt.bfloat16
    fp8 = mybir.dt.float8e4
    nc = bacc.Bacc(
        "TRN2",
        target_bir_lowering=False,
        debug=False,
        enable_asserts=False,
        num_devices=NCORES,
    )
    sup = nc.dram_tensor(
        "sup", [128, OHW + SCH * D], fp8, kind="ExternalInput"
    ).ap()
    qt = nc.dram_tensor("qt", [128, DCH * QL], bf16, kind="ExternalInput").ap()
    out = nc.dram_tensor("out", [CL, QL], f32, kind="ExternalOutput").ap()
    with tile.TileContext(nc) as tc:
        _emit(nc, tc, sup, qt, out)
    nc.compile()
    _CACHE["nc"] = nc
    return nc


def _onehot():
    import ml_dtypes

    # oh[p, jp, o, c] = 1 iff class c owns support row (2jp+o)*128+p,
    # i.e. c == 4jp + 2o + p//64
    p = np.arange(128)[:, None, None, None]
    jp = np.arange(NJP)[None, :, None, None]
    o = np.arange(2)[None, None, :, None]
    c = np.arange(CL)[None, None, None, :]
    oh = (c == 4 * jp + 2 * o + p // 64).astype(ml_dtypes.float8_e4m3)
    return oh.reshape(128, OHW)


def _shard(embeddings):
    import ml_dtypes

    emb = np.asarray(embeddings, dtype=np.float32).reshape(C, 2 * S, D)
    oh = _onehot()
    # support per class-group b: one-hot header + swizzled [128, SCH, D]
    sups = []
    for b in range(BC):
        sb = emb[CL * b : CL * (b + 1), :S, :].reshape(SCH, 128, D)
        sb = sb.transpose(1, 0, 2).astype(ml_dtypes.float8_e4m3)
        sups.append(
            np.ascontiguousarray(
                np.concatenate([oh, sb.reshape(128, SCH * D)], axis=1)
            )
        )
    # queries per query-group a: Q^T swizzled [128, DCH, QL] bf16
    query_set = emb[:, S:, :].reshape(C * S, D)
    qts = []
    for a in range(AQ):
        q = query_set[QL * a : QL * (a + 1)]
        qt_a = q.T.reshape(DCH, 128, QL).transpose(1, 0, 2)
        qts.append(
            np.ascontiguousarray(
                qt_a.astype(ml_dtypes.bfloat16).reshape(128, DCH * QL)
            )
        )
    in_maps = []
    for i in range(NCORES):
        a, b = divmod(i, BC)
        in_maps.append({"sup": sups[b], "qt": qts[a]})
    return in_maps


def _gather(outs):
    """Stitch per-core [CL, QL] blocks into full [C*S, C] logits."""
    logits = np.empty((C * S, C), dtype=np.float32)
    for i in range(NCORES):
        a, b = divmod(i, BC)
        logits[QL * a : QL * (a + 1), CL * b : CL * (b + 1)] = (
            np.asarray(outs[i], dtype=np.float32).T
        )
    return logits


def kernel(embeddings_stacked, n_classes, n_support, **_unused):
    assert int(n_classes) == C and int(n_support) == S
    emb = np.asarray(embeddings_stacked)
    assert emb.shape == (C * 2 * S, D), emb.shape

    from concourse import bass_utils

    nc = _build()
    in_maps = _shard(emb)
    try:
        res = bass_utils.run_bass_kernel_spmd(
            nc, in_maps, core_ids=list(range(NCORES))
        )
    except Exception:
        # transient device/runtime hiccups have been observed; retry once
        res = bass_utils.run_bass_kernel_spmd(
            nc, in_maps, core_ids=list(range(NCORES))
        )
    return _gather([res.results[i]["out"] for i in range(NCORES)])


if __name__ == "__main__":
    rng = np.random.default_rng(0)
    emb = rng.standard_normal((C * 2 * S, D), dtype=np.float32)
    got = kernel(emb, C, S)
    print("kernel output", got.shape, got.dtype)
